# revision 13
# baseline (speedup 1.0000x reference)
"""Trainium2 Bass kernel for nn_BilinearFeedForward.

reference (B=4, N=2048, D=1024, fp32):
    query = (x_real @ Wqr) * (x_imag @ Wqi)            # [B,N,D]
    key   = x_real @ Wk ; value = x_imag @ Wv          # [B,N,D]
    key   /= max(||key||_n, eps) ; value /= max(||value||_n, eps)   (norm over N)
    kv    = einsum('bnd,bne->bde', key, value)         # [B,D,D]
    out   = einsum('bnd,bde->bne', query, kv) + bias   # [B,N,D]

Sharding: 8 cores = (batch b in 0..3) x (e-half eh in 0..1).  Each core
computes out[b, :, eh*512:(eh+1)*512] with zero collectives:
  - full-width K, Qr, Qi for its batch, half-width V
  - normalization folded into scalars:  kv = diag(1/sk) (K^T V) diag(1/sv)
  - everything runs transposed so per-feature scalars live on partitions.

Host pre-transposes x to x.T per batch (feeds the PE contraction layout)
and re-transposes the per-core [512, 2048] outputs.
"""

import os
import sys
import numpy as np

for _p in ("/opt/trn_rl_repo", "/root/.axon_site/_ro/trn_rl_repo"):
    if _p not in sys.path and os.path.isdir(_p):
        sys.path.append(_p)

# Some images lack antenv.axon_hooks; bass_utils imports it unconditionally
# when BASS_TRACE is set.  Provide a degrade-to-no-trace shim if missing.
try:
    import antenv.axon_hooks  # noqa: F401
except Exception:
    import types

    try:
        import antenv

        _hooks = types.ModuleType("antenv.axon_hooks")
        _hooks._hook = None
        _hooks.get_axon_ntff_profile_hook = lambda: _hooks._hook

        def _set_hook(h):
            _hooks._hook = h

        _hooks.set_axon_ntff_profile_hook = _set_hook
        sys.modules["antenv.axon_hooks"] = _hooks
        antenv.axon_hooks = _hooks
    except Exception:
        pass

B, N, D = 4, 2048, 1024
EH = 512          # e-half width
P = 128
DT = D // P       # 8 d-tiles
ET = EH // P      # 4 e-tiles of the half
NCK = 512         # n-chunk
CHUNKS = N // NCK # 4
EPS = 1e-5

_CACHE = {}
LAST_EXEC_NS = None


def _build_bass():
    import concourse.bacc as bacc
    import concourse.tile as tile
    import concourse.mybir as mybir

    f32 = mybir.dt.float32
    f32r = mybir.dt.float32r
    bf16 = mybir.dt.bfloat16
    Act = mybir.ActivationFunctionType
    Alu = mybir.AluOpType

    nc = bacc.Bacc()

    xrt_d = nc.dram_tensor("xrt", [D, N], f32, kind="ExternalInput")
    xit_d = nc.dram_tensor("xit", [D, N], f32, kind="ExternalInput")
    wqr_d = nc.dram_tensor("wqr", [D, D], f32, kind="ExternalInput")
    wqi_d = nc.dram_tensor("wqi", [D, D], f32, kind="ExternalInput")
    wk_d = nc.dram_tensor("wk", [D, D], f32, kind="ExternalInput")
    wv_d = nc.dram_tensor("wv", [D, EH], f32, kind="ExternalInput")
    bias_d = nc.dram_tensor("bias", [EH], f32, kind="ExternalInput")
    out_d = nc.dram_tensor("out_t", [EH, N], f32, kind="ExternalOutput")

    xrt_r = xrt_d.rearrange("(t p) n -> p t n", p=P)
    xit_r = xit_d.rearrange("(t p) n -> p t n", p=P)
    wqr_r = wqr_d.rearrange("(t p) e -> p t e", p=P)
    wqi_r = wqi_d.rearrange("(t p) e -> p t e", p=P)
    wk_r = wk_d.rearrange("(t p) e -> p t e", p=P)
    wv_r = wv_d.rearrange("(t p) e -> p t e", p=P)
    bias_r = bias_d.rearrange("(t p) -> p t", p=P)
    out_r = out_d.rearrange("(t p) n -> p t n", p=P)

    with tile.TileContext(nc) as tc:
        with tc.tile_pool(name="outer", bufs=1) as outer:
            a_sb = outer.tile([P, DT, EH], f32, tag="a_sb")
            skinv = outer.tile([P, DT], f32, tag="skinv")
            svinv = outer.tile([P, ET], f32, tag="svinv")
            bias_pp = outer.tile([P, ET], f32, tag="bias_pp")
            zero32 = outer.tile([P, 1], f32, tag="zero32")
            ones_bf = outer.tile([P, 1], bf16, tag="ones_bf")
            nc.vector.memset(zero32[:], 0.0)
            nc.vector.memset(ones_bf[:], 1.0)
            nc.sync.dma_start(out=bias_pp[:], in_=bias_r)

            sk2_sb = outer.tile([P, DT], f32, tag="sk2_sb")
            sv2_sb = outer.tile([P, ET], f32, tag="sv2_sb")
            nc.vector.memset(sk2_sb[:], 0.0)
            nc.vector.memset(sv2_sb[:], 0.0)

            # ---------------- Phase A: K, V, A = K^T V, norms ----------------
            NCKA = 256
            CHA = N // NCKA
            NTA = NCKA // P
            ctx_wq = tc.tile_pool(name="wq", bufs=1)
            wq = ctx_wq.__enter__()
            wqr_sb = wq.tile([P, DT, D], f32r, tag="wqr")
            wqi_sb = wq.tile([P, DT, D], f32r, tag="wqi")
            with (
                tc.tile_pool(name="wkv", bufs=1) as wkv,
                tc.tile_pool(name="xin", bufs=2) as xin,
                tc.tile_pool(name="kv", bufs=2) as kvp,
                tc.tile_pool(name="sqp", bufs=3) as sqp,
                tc.tile_pool(name="ps_kv", bufs=3, space="PSUM") as ps_kv,
                tc.tile_pool(name="ps_a", bufs=2, space="PSUM") as ps_a,
                tc.tile_pool(name="nrm_ps", bufs=3, space="PSUM") as nrm_ps,
            ):
                wk_sb = wkv.tile([P, DT, D], f32r, tag="wk")
                wv_sb = wkv.tile([P, DT, EH], f32r, tag="wv")

                for c4 in range(CHA):
                    ns = c4 * NCKA
                    xrt_c = xin.tile([P, DT, NCKA], f32r, tag="xrt_c")
                    xit_c = xin.tile([P, DT, NCKA], f32r, tag="xit_c")
                    if c4 == 0:
                        # fine-grained startup: pair each x slice with the
                        # weight slice the first accumulation step needs
                        for t in range(DT):
                            nc.sync.dma_start(out=xrt_c[:, t, :],
                                              in_=xrt_r[:, t, ns:ns + NCKA].bitcast(f32r))
                            nc.sync.dma_start(out=wk_sb[:, t, :],
                                              in_=wk_r[:, t, :].bitcast(f32r))
                            nc.gpsimd.dma_start(out=xit_c[:, t, :],
                                                in_=xit_r[:, t, ns:ns + NCKA].bitcast(f32r))
                            nc.gpsimd.dma_start(out=wv_sb[:, t, :],
                                                in_=wv_r[:, t, :].bitcast(f32r))
                    else:
                        nc.sync.dma_start(out=xrt_c[:], in_=xrt_r[:, :, ns:ns + NCKA].bitcast(f32r))
                        nc.gpsimd.dma_start(out=xit_c[:], in_=xit_r[:, :, ns:ns + NCKA].bitcast(f32r))
                    if c4 == 3:
                        # prefetch phase-C weights in the DMA slack mid-phase
                        for t in range(DT):
                            nc.sync.dma_start(out=wqr_sb[:, t, :],
                                              in_=wqr_r[:, t, :].bitcast(f32r))
                            nc.gpsimd.dma_start(out=wqi_sb[:, t, :],
                                                in_=wqi_r[:, t, :].bitcast(f32r))

                    k_c = kvp.tile([P, NTA, D], f32r, tag="k_c")
                    v_c = kvp.tile([P, NTA, EH], f32r, tag="v_c")

                    first = c4 == 0
                    for nt in range(NTA):
                        nsl = slice(nt * P, (nt + 1) * P)
                        for do2 in range(2):
                            kps = ps_kv.tile([P, 512], f32, tag="kvps")
                            for t in range(DT):
                                nc.tensor.matmul(
                                    kps[:], xrt_c[:, t, nsl],
                                    wk_sb[:, t, do2 * 512:(do2 + 1) * 512],
                                    start=(t == 0), stop=(t == DT - 1))
                            nc.vector.tensor_copy(
                                out=k_c[:, nt, do2 * 512:(do2 + 1) * 512], in_=kps[:])
                            sq = sqp.tile([P, 512], bf16, tag="sq")
                            nc.scalar.activation(out=sq[:], in_=kps[:], func=Act.Square,
                                                 bias=zero32[:], scale=1.0)
                            nps = nrm_ps.tile([P, 4], f32, tag="nps")
                            for j in range(4):
                                nc.tensor.matmul(
                                    nps[:, j:j + 1], sq[:, j * P:(j + 1) * P],
                                    ones_bf[:], start=True, stop=True)
                            nc.vector.tensor_add(
                                out=sk2_sb[:, do2 * 4:(do2 + 1) * 4],
                                in0=sk2_sb[:, do2 * 4:(do2 + 1) * 4], in1=nps[:])
                        # V rows [128n, 512]
                        vps = ps_kv.tile([P, 512], f32, tag="kvps")
                        for t in range(DT):
                            nc.tensor.matmul(vps[:], xit_c[:, t, nsl], wv_sb[:, t, :],
                                             start=(t == 0), stop=(t == DT - 1))
                        nc.vector.tensor_copy(out=v_c[:, nt, :], in_=vps[:])
                        sqv = sqp.tile([P, 512], bf16, tag="sq")
                        nc.scalar.activation(out=sqv[:], in_=vps[:], func=Act.Square,
                                             bias=zero32[:], scale=1.0)
                        npsv = nrm_ps.tile([P, 4], f32, tag="nps")
                        for j in range(4):
                            nc.tensor.matmul(
                                npsv[:, j:j + 1], sqv[:, j * P:(j + 1) * P],
                                ones_bf[:], start=True, stop=True)
                        nc.vector.tensor_add(out=sv2_sb[:], in0=sv2_sb[:], in1=npsv[:])

                    # A += K_c^T @ V_c  (contract the chunk's rows)
                    for dt in range(DT):
                        aps = ps_a.tile([P, EH], f32, tag="aps")
                        for nt in range(NTA):
                            nc.tensor.matmul(aps[:], k_c[:, nt, dt * P:(dt + 1) * P],
                                             v_c[:, nt, :], start=(nt == 0),
                                             stop=(nt == NTA - 1))
                        if first:
                            nc.vector.tensor_copy(out=a_sb[:, dt, :].bitcast(f32r),
                                                  in_=aps[:])
                        else:
                            nc.vector.tensor_add(out=a_sb[:, dt, :].bitcast(f32r),
                                                 in0=a_sb[:, dt, :], in1=aps[:])

            # ---------------- Phase B: finalize norms, scale A ----------------
            nc.scalar.activation(out=skinv[:], in_=sk2_sb[:], func=Act.Sqrt,
                                 bias=zero32[:], scale=1.0)
            nc.vector.tensor_scalar_max(skinv[:], skinv[:], EPS)
            nc.vector.reciprocal(skinv[:], skinv[:])
            nc.scalar.activation(out=svinv[:], in_=sv2_sb[:], func=Act.Sqrt,
                                 bias=zero32[:], scale=1.0)
            nc.vector.tensor_scalar_max(svinv[:], svinv[:], EPS)
            nc.vector.reciprocal(svinv[:], svinv[:])
            for dt in range(DT):
                nc.vector.tensor_scalar_mul(
                    out=a_sb[:, dt, :].bitcast(f32r), in0=a_sb[:, dt, :],
                    scalar1=skinv[:, dt:dt + 1])

            # ---------------- Phase C: Q^T and out^T = (A')^T Q^T ------------
            with (
                tc.tile_pool(name="xin2", bufs=2) as xin2,
                tc.tile_pool(name="qrp", bufs=3) as qrp,
                tc.tile_pool(name="qtp", bufs=2) as qtp,
                tc.tile_pool(name="outp", bufs=3) as outp,
                tc.tile_pool(name="ps_q", bufs=4, space="PSUM") as ps_q,
                tc.tile_pool(name="ps_o", bufs=2, space="PSUM") as ps_o,
            ):
                for c4 in range(CHUNKS):
                    ns = c4 * NCK
                    xrt_c = xin2.tile([P, DT, NCK], f32r, tag="xrt_c2")
                    xit_c = xin2.tile([P, DT, NCK], f32r, tag="xit_c2")
                    nc.sync.dma_start(out=xrt_c[:], in_=xrt_r[:, :, ns:ns + NCK].bitcast(f32r))
                    nc.gpsimd.dma_start(out=xit_c[:], in_=xit_r[:, :, ns:ns + NCK].bitcast(f32r))

                    qt_c = qtp.tile([P, DT, NCK], f32r, tag="qt_c")
                    for dqt in range(DT):
                        qsl = slice(dqt * P, (dqt + 1) * P)
                        qrps = ps_q.tile([P, NCK], f32, tag="qps")
                        for t in range(DT):
                            nc.tensor.matmul(qrps[:], wqr_sb[:, t, qsl], xrt_c[:, t, :],
                                             start=(t == 0), stop=(t == DT - 1))
                        qr_sb = qrp.tile([P, NCK], f32, tag="qr_sb")
                        nc.scalar.activation(out=qr_sb[:], in_=qrps[:], func=Act.Copy,
                                             bias=0.0, scale=1.0)
                        qips = ps_q.tile([P, NCK], f32, tag="qps")
                        for t in range(DT):
                            nc.tensor.matmul(qips[:], wqi_sb[:, t, qsl], xit_c[:, t, :],
                                             start=(t == 0), stop=(t == DT - 1))
                        nc.vector.tensor_mul(out=qt_c[:, dqt, :], in0=qips[:], in1=qr_sb[:])

                    for et in range(ET):
                        esl = slice(et * P, (et + 1) * P)
                        ops_t = ps_o.tile([P, NCK], f32, tag="ops")
                        for dt in range(DT):
                            nc.tensor.matmul(ops_t[:], a_sb[:, dt, esl].bitcast(f32r),
                                             qt_c[:, dt, :],
                                             start=(dt == 0), stop=(dt == DT - 1))
                        out_sb = outp.tile([P, NCK], f32, tag="out_sb")
                        nc.vector.tensor_scalar(
                            out=out_sb[:], in0=ops_t[:],
                            scalar1=svinv[:, et:et + 1], scalar2=bias_pp[:, et:et + 1],
                            op0=Alu.mult, op1=Alu.add)
                        nc.sync.dma_start(out=out_r[:, et, ns:ns + NCK], in_=out_sb[:])

            ctx_wq.__exit__(None, None, None)

    nc.finalize()
    return nc


def kernel(x_real, x_imag, w_query_real, w_query_imag, w_key, w_value, bias):
    global LAST_EXEC_NS
    from concourse.bass_utils import run_bass_kernel_spmd

    x_real = np.ascontiguousarray(np.asarray(x_real, dtype=np.float32))
    x_imag = np.ascontiguousarray(np.asarray(x_imag, dtype=np.float32))
    wqr = np.ascontiguousarray(np.asarray(w_query_real, dtype=np.float32))
    wqi = np.ascontiguousarray(np.asarray(w_query_imag, dtype=np.float32))
    wk = np.ascontiguousarray(np.asarray(w_key, dtype=np.float32))
    wv = np.ascontiguousarray(np.asarray(w_value, dtype=np.float32))
    bias = np.ascontiguousarray(np.asarray(bias, dtype=np.float32))

    nc = _CACHE.get("nc")
    if nc is None:
        nc = _build_bass()
        _CACHE["nc"] = nc

    xrt = [np.ascontiguousarray(x_real[b].T) for b in range(B)]
    xit = [np.ascontiguousarray(x_imag[b].T) for b in range(B)]
    wv_h = [np.ascontiguousarray(wv[:, eh * EH:(eh + 1) * EH]) for eh in range(2)]
    bias_h = [np.ascontiguousarray(bias[eh * EH:(eh + 1) * EH]) for eh in range(2)]

    in_maps = []
    for c in range(8):
        b, eh = c // 2, c % 2
        in_maps.append({
            "xrt": xrt[b], "xit": xit[b],
            "wqr": wqr, "wqi": wqi, "wk": wk,
            "wv": wv_h[eh], "bias": bias_h[eh],
        })

    res = run_bass_kernel_spmd(nc, in_maps, list(range(8)))
    LAST_EXEC_NS = res.exec_time_ns

    out = np.empty((B, N, D), dtype=np.float32)
    for c in range(8):
        b, eh = c // 2, c % 2
        out[b, :, eh * EH:(eh + 1) * EH] = np.asarray(res.results[c]["out_t"]).T
    return out


# revision 14
# speedup vs baseline: 1.0190x; 1.0190x over previous
"""Trainium2 Bass kernel for nn_BilinearFeedForward.

reference (B=4, N=2048, D=1024, fp32):
    query = (x_real @ Wqr) * (x_imag @ Wqi)            # [B,N,D]
    key   = x_real @ Wk ; value = x_imag @ Wv          # [B,N,D]
    key   /= max(||key||_n, eps) ; value /= max(||value||_n, eps)   (norm over N)
    kv    = einsum('bnd,bne->bde', key, value)         # [B,D,D]
    out   = einsum('bnd,bde->bne', query, kv) + bias   # [B,N,D]

Sharding: 8 cores = (batch b in 0..3) x (e-half eh in 0..1).  Each core
computes out[b, :, eh*512:(eh+1)*512] with zero collectives:
  - full-width K, Qr, Qi for its batch, half-width V
  - normalization folded into scalars:  kv = diag(1/sk) (K^T V) diag(1/sv)
  - everything runs transposed so per-feature scalars live on partitions.

Host pre-transposes x to x.T per batch (feeds the PE contraction layout)
and re-transposes the per-core [512, 2048] outputs.
"""

import os
import sys
import numpy as np

for _p in ("/opt/trn_rl_repo", "/root/.axon_site/_ro/trn_rl_repo"):
    if _p not in sys.path and os.path.isdir(_p):
        sys.path.append(_p)

# Some images lack antenv.axon_hooks; bass_utils imports it unconditionally
# when BASS_TRACE is set.  Provide a degrade-to-no-trace shim if missing.
try:
    import antenv.axon_hooks  # noqa: F401
except Exception:
    import types

    try:
        import antenv

        _hooks = types.ModuleType("antenv.axon_hooks")
        _hooks._hook = None
        _hooks.get_axon_ntff_profile_hook = lambda: _hooks._hook

        def _set_hook(h):
            _hooks._hook = h

        _hooks.set_axon_ntff_profile_hook = _set_hook
        sys.modules["antenv.axon_hooks"] = _hooks
        antenv.axon_hooks = _hooks
    except Exception:
        pass

B, N, D = 4, 2048, 1024
EH = 512          # e-half width
P = 128
DT = D // P       # 8 d-tiles
ET = EH // P      # 4 e-tiles of the half
NCK = 512         # n-chunk
CHUNKS = N // NCK # 4
EPS = 1e-5

_CACHE = {}
LAST_EXEC_NS = None


def _build_bass():
    import concourse.bacc as bacc
    import concourse.tile as tile
    import concourse.mybir as mybir

    f32 = mybir.dt.float32
    f32r = mybir.dt.float32r
    bf16 = mybir.dt.bfloat16
    Act = mybir.ActivationFunctionType
    Alu = mybir.AluOpType

    nc = bacc.Bacc()

    xrt_d = nc.dram_tensor("xrt", [D, N], f32, kind="ExternalInput")
    xit_d = nc.dram_tensor("xit", [D, N], f32, kind="ExternalInput")
    wqr_d = nc.dram_tensor("wqr", [D, D], f32, kind="ExternalInput")
    wqi_d = nc.dram_tensor("wqi", [D, D], f32, kind="ExternalInput")
    wk_d = nc.dram_tensor("wk", [D, D], f32, kind="ExternalInput")
    wv_d = nc.dram_tensor("wv", [D, EH], f32, kind="ExternalInput")
    bias_d = nc.dram_tensor("bias", [EH], f32, kind="ExternalInput")
    out_d = nc.dram_tensor("out_t", [EH, N], f32, kind="ExternalOutput")

    xrt_r = xrt_d.rearrange("(t p) n -> p t n", p=P)
    xit_r = xit_d.rearrange("(t p) n -> p t n", p=P)
    wqr_r = wqr_d.rearrange("(t p) e -> p t e", p=P)
    wqi_r = wqi_d.rearrange("(t p) e -> p t e", p=P)
    wk_r = wk_d.rearrange("(t p) e -> p t e", p=P)
    wv_r = wv_d.rearrange("(t p) e -> p t e", p=P)
    bias_r = bias_d.rearrange("(t p) -> p t", p=P)
    out_r = out_d.rearrange("(t p) n -> p t n", p=P)

    with tile.TileContext(nc) as tc:
        with tc.tile_pool(name="outer", bufs=1) as outer:
            a_sb = outer.tile([P, DT, EH], f32, tag="a_sb")
            skinv = outer.tile([P, DT], f32, tag="skinv")
            svinv = outer.tile([P, ET], f32, tag="svinv")
            bias_pp = outer.tile([P, ET], f32, tag="bias_pp")
            zero32 = outer.tile([P, 1], f32, tag="zero32")
            ones_bf = outer.tile([P, 1], bf16, tag="ones_bf")
            nc.vector.memset(zero32[:], 0.0)
            nc.vector.memset(ones_bf[:], 1.0)
            nc.sync.dma_start(out=bias_pp[:], in_=bias_r)

            sk2_sb = outer.tile([P, DT], f32, tag="sk2_sb")
            sv2_sb = outer.tile([P, ET], f32, tag="sv2_sb")
            nc.vector.memset(sk2_sb[:], 0.0)
            nc.vector.memset(sv2_sb[:], 0.0)

            # ---------------- Phase A: K, V, A = K^T V, norms ----------------
            NCKA = 256
            CHA = N // NCKA
            NTA = NCKA // P
            ctx_wq = tc.tile_pool(name="wq", bufs=1)
            wq = ctx_wq.__enter__()
            wqr_sb = wq.tile([P, DT, D], f32r, tag="wqr")
            wqi_sb = wq.tile([P, DT, D], f32r, tag="wqi")
            with (
                tc.tile_pool(name="wkv", bufs=1) as wkv,
                tc.tile_pool(name="xin", bufs=2) as xin,
                tc.tile_pool(name="kv", bufs=2) as kvp,
                tc.tile_pool(name="sqp", bufs=3) as sqp,
                tc.tile_pool(name="ps_kv", bufs=4, space="PSUM") as ps_kv,
                tc.tile_pool(name="ps_a", bufs=2, space="PSUM") as ps_a,
                tc.tile_pool(name="nrm_ps", bufs=2, space="PSUM") as nrm_ps,
            ):
                wk_sb = wkv.tile([P, DT, D], f32r, tag="wk")
                wv_sb = wkv.tile([P, DT, EH], f32r, tag="wv")

                for c4 in range(CHA):
                    ns = c4 * NCKA
                    xrt_c = xin.tile([P, DT, NCKA], f32r, tag="xrt_c")
                    xit_c = xin.tile([P, DT, NCKA], f32r, tag="xit_c")
                    if c4 == 0:
                        # startup: split the K-path inputs (xrt + wk) across
                        # both DMA queues so the first chains unblock fastest
                        nc.sync.dma_start(out=xrt_c[:],
                                          in_=xrt_r[:, :, ns:ns + NCKA].bitcast(f32r))
                        for t in range(DT):
                            eng = nc.sync if t % 2 else nc.gpsimd
                            eng.dma_start(out=wk_sb[:, t, :],
                                          in_=wk_r[:, t, :].bitcast(f32r))
                        nc.gpsimd.dma_start(out=xit_c[:],
                                            in_=xit_r[:, :, ns:ns + NCKA].bitcast(f32r))
                        for t in range(DT):
                            eng = nc.gpsimd if t % 2 else nc.sync
                            eng.dma_start(out=wv_sb[:, t, :],
                                          in_=wv_r[:, t, :].bitcast(f32r))
                    else:
                        nc.sync.dma_start(out=xrt_c[:], in_=xrt_r[:, :, ns:ns + NCKA].bitcast(f32r))
                        nc.gpsimd.dma_start(out=xit_c[:], in_=xit_r[:, :, ns:ns + NCKA].bitcast(f32r))
                    if c4 == 3:
                        # prefetch phase-C weights in the DMA slack mid-phase
                        for t in range(DT):
                            nc.sync.dma_start(out=wqr_sb[:, t, :],
                                              in_=wqr_r[:, t, :].bitcast(f32r))
                            nc.gpsimd.dma_start(out=wqi_sb[:, t, :],
                                                in_=wqi_r[:, t, :].bitcast(f32r))

                    k_c = kvp.tile([P, NTA, D], f32r, tag="k_c")
                    v_c = kvp.tile([P, NTA, EH], f32r, tag="v_c")

                    first = c4 == 0
                    for nt in range(NTA):
                        nsl = slice(nt * P, (nt + 1) * P)
                        for do2 in range(2):
                            kps = ps_kv.tile([P, 512], f32, tag="kvps")
                            for t in range(DT):
                                nc.tensor.matmul(
                                    kps[:], xrt_c[:, t, nsl],
                                    wk_sb[:, t, do2 * 512:(do2 + 1) * 512],
                                    start=(t == 0), stop=(t == DT - 1))
                            nc.vector.tensor_copy(
                                out=k_c[:, nt, do2 * 512:(do2 + 1) * 512], in_=kps[:])
                            sq = sqp.tile([P, 512], bf16, tag="sq")
                            nc.scalar.activation(out=sq[:], in_=kps[:], func=Act.Square,
                                                 bias=zero32[:], scale=1.0)
                            nps = nrm_ps.tile([P, 4], f32, tag="nps")
                            for j in range(4):
                                nc.tensor.matmul(
                                    nps[:, j:j + 1], sq[:, j * P:(j + 1) * P],
                                    ones_bf[:], start=True, stop=True)
                            nc.vector.tensor_add(
                                out=sk2_sb[:, do2 * 4:(do2 + 1) * 4],
                                in0=sk2_sb[:, do2 * 4:(do2 + 1) * 4], in1=nps[:])
                        # V rows [128n, 512]
                        vps = ps_kv.tile([P, 512], f32, tag="kvps")
                        for t in range(DT):
                            nc.tensor.matmul(vps[:], xit_c[:, t, nsl], wv_sb[:, t, :],
                                             start=(t == 0), stop=(t == DT - 1))
                        nc.vector.tensor_copy(out=v_c[:, nt, :], in_=vps[:])
                        sqv = sqp.tile([P, 512], bf16, tag="sq")
                        nc.scalar.activation(out=sqv[:], in_=vps[:], func=Act.Square,
                                             bias=zero32[:], scale=1.0)
                        npsv = nrm_ps.tile([P, 4], f32, tag="nps")
                        for j in range(4):
                            nc.tensor.matmul(
                                npsv[:, j:j + 1], sqv[:, j * P:(j + 1) * P],
                                ones_bf[:], start=True, stop=True)
                        nc.vector.tensor_add(out=sv2_sb[:], in0=sv2_sb[:], in1=npsv[:])

                    # A += K_c^T @ V_c  (contract the chunk's rows)
                    for dt in range(DT):
                        aps = ps_a.tile([P, EH], f32, tag="aps")
                        for nt in range(NTA):
                            nc.tensor.matmul(aps[:], k_c[:, nt, dt * P:(dt + 1) * P],
                                             v_c[:, nt, :], start=(nt == 0),
                                             stop=(nt == NTA - 1))
                        if first:
                            nc.vector.tensor_copy(out=a_sb[:, dt, :].bitcast(f32r),
                                                  in_=aps[:])
                        else:
                            nc.vector.tensor_add(out=a_sb[:, dt, :].bitcast(f32r),
                                                 in0=a_sb[:, dt, :], in1=aps[:])

            # ---------------- Phase B: finalize norms, scale A ----------------
            nc.scalar.activation(out=skinv[:], in_=sk2_sb[:], func=Act.Sqrt,
                                 bias=zero32[:], scale=1.0)
            nc.vector.tensor_scalar_max(skinv[:], skinv[:], EPS)
            nc.vector.reciprocal(skinv[:], skinv[:])
            nc.scalar.activation(out=svinv[:], in_=sv2_sb[:], func=Act.Sqrt,
                                 bias=zero32[:], scale=1.0)
            nc.vector.tensor_scalar_max(svinv[:], svinv[:], EPS)
            nc.vector.reciprocal(svinv[:], svinv[:])
            for dt in range(DT):
                nc.vector.tensor_scalar_mul(
                    out=a_sb[:, dt, :].bitcast(f32r), in0=a_sb[:, dt, :],
                    scalar1=skinv[:, dt:dt + 1])

            # ---------------- Phase C: Q^T and out^T = (A')^T Q^T ------------
            with (
                tc.tile_pool(name="xin2", bufs=2) as xin2,
                tc.tile_pool(name="qrp", bufs=3) as qrp,
                tc.tile_pool(name="qtp", bufs=2) as qtp,
                tc.tile_pool(name="outp", bufs=3) as outp,
                tc.tile_pool(name="ps_q", bufs=4, space="PSUM") as ps_q,
                tc.tile_pool(name="ps_o", bufs=2, space="PSUM") as ps_o,
            ):
                for c4 in range(CHUNKS):
                    ns = c4 * NCK
                    xrt_c = xin2.tile([P, DT, NCK], f32r, tag="xrt_c2")
                    xit_c = xin2.tile([P, DT, NCK], f32r, tag="xit_c2")
                    nc.sync.dma_start(out=xrt_c[:], in_=xrt_r[:, :, ns:ns + NCK].bitcast(f32r))
                    nc.gpsimd.dma_start(out=xit_c[:], in_=xit_r[:, :, ns:ns + NCK].bitcast(f32r))

                    qt_c = qtp.tile([P, DT, NCK], f32r, tag="qt_c")
                    for dqt in range(DT):
                        qsl = slice(dqt * P, (dqt + 1) * P)
                        qrps = ps_q.tile([P, NCK], f32, tag="qps")
                        for t in range(DT):
                            nc.tensor.matmul(qrps[:], wqr_sb[:, t, qsl], xrt_c[:, t, :],
                                             start=(t == 0), stop=(t == DT - 1))
                        qr_sb = qrp.tile([P, NCK], f32, tag="qr_sb")
                        nc.scalar.activation(out=qr_sb[:], in_=qrps[:], func=Act.Copy,
                                             bias=0.0, scale=1.0)
                        qips = ps_q.tile([P, NCK], f32, tag="qps")
                        for t in range(DT):
                            nc.tensor.matmul(qips[:], wqi_sb[:, t, qsl], xit_c[:, t, :],
                                             start=(t == 0), stop=(t == DT - 1))
                        nc.vector.tensor_mul(out=qt_c[:, dqt, :], in0=qips[:], in1=qr_sb[:])

                    for et in range(ET):
                        esl = slice(et * P, (et + 1) * P)
                        ops_t = ps_o.tile([P, NCK], f32, tag="ops")
                        for dt in range(DT):
                            nc.tensor.matmul(ops_t[:], a_sb[:, dt, esl].bitcast(f32r),
                                             qt_c[:, dt, :],
                                             start=(dt == 0), stop=(dt == DT - 1))
                        out_sb = outp.tile([P, NCK], f32, tag="out_sb")
                        nc.vector.tensor_scalar(
                            out=out_sb[:], in0=ops_t[:],
                            scalar1=svinv[:, et:et + 1], scalar2=bias_pp[:, et:et + 1],
                            op0=Alu.mult, op1=Alu.add)
                        nc.sync.dma_start(out=out_r[:, et, ns:ns + NCK], in_=out_sb[:])

            ctx_wq.__exit__(None, None, None)

    nc.finalize()
    return nc


def kernel(x_real, x_imag, w_query_real, w_query_imag, w_key, w_value, bias):
    global LAST_EXEC_NS
    from concourse.bass_utils import run_bass_kernel_spmd

    x_real = np.ascontiguousarray(np.asarray(x_real, dtype=np.float32))
    x_imag = np.ascontiguousarray(np.asarray(x_imag, dtype=np.float32))
    wqr = np.ascontiguousarray(np.asarray(w_query_real, dtype=np.float32))
    wqi = np.ascontiguousarray(np.asarray(w_query_imag, dtype=np.float32))
    wk = np.ascontiguousarray(np.asarray(w_key, dtype=np.float32))
    wv = np.ascontiguousarray(np.asarray(w_value, dtype=np.float32))
    bias = np.ascontiguousarray(np.asarray(bias, dtype=np.float32))

    nc = _CACHE.get("nc")
    if nc is None:
        nc = _build_bass()
        _CACHE["nc"] = nc

    xrt = [np.ascontiguousarray(x_real[b].T) for b in range(B)]
    xit = [np.ascontiguousarray(x_imag[b].T) for b in range(B)]
    wv_h = [np.ascontiguousarray(wv[:, eh * EH:(eh + 1) * EH]) for eh in range(2)]
    bias_h = [np.ascontiguousarray(bias[eh * EH:(eh + 1) * EH]) for eh in range(2)]

    in_maps = []
    for c in range(8):
        b, eh = c // 2, c % 2
        in_maps.append({
            "xrt": xrt[b], "xit": xit[b],
            "wqr": wqr, "wqi": wqi, "wk": wk,
            "wv": wv_h[eh], "bias": bias_h[eh],
        })

    res = run_bass_kernel_spmd(nc, in_maps, list(range(8)))
    LAST_EXEC_NS = res.exec_time_ns

    out = np.empty((B, N, D), dtype=np.float32)
    for c in range(8):
        b, eh = c // 2, c % 2
        out[b, :, eh * EH:(eh + 1) * EH] = np.asarray(res.results[c]["out_t"]).T
    return out


# revision 15
# speedup vs baseline: 1.0261x; 1.0070x over previous
"""Trainium2 Bass kernel for nn_BilinearFeedForward.

reference (B=4, N=2048, D=1024, fp32):
    query = (x_real @ Wqr) * (x_imag @ Wqi)            # [B,N,D]
    key   = x_real @ Wk ; value = x_imag @ Wv          # [B,N,D]
    key   /= max(||key||_n, eps) ; value /= max(||value||_n, eps)   (norm over N)
    kv    = einsum('bnd,bne->bde', key, value)         # [B,D,D]
    out   = einsum('bnd,bde->bne', query, kv) + bias   # [B,N,D]

Sharding: 8 cores = (batch b in 0..3) x (e-half eh in 0..1).  Each core
computes out[b, :, eh*512:(eh+1)*512] with zero collectives:
  - full-width K, Qr, Qi for its batch, half-width V
  - normalization folded into scalars:  kv = diag(1/sk) (K^T V) diag(1/sv)
  - everything runs transposed so per-feature scalars live on partitions.

Host pre-transposes x to x.T per batch (feeds the PE contraction layout)
and re-transposes the per-core [512, 2048] outputs.
"""

import os
import sys
import numpy as np

for _p in ("/opt/trn_rl_repo", "/root/.axon_site/_ro/trn_rl_repo"):
    if _p not in sys.path and os.path.isdir(_p):
        sys.path.append(_p)

# Some images lack antenv.axon_hooks; bass_utils imports it unconditionally
# when BASS_TRACE is set.  Provide a degrade-to-no-trace shim if missing.
try:
    import antenv.axon_hooks  # noqa: F401
except Exception:
    import types

    try:
        import antenv

        _hooks = types.ModuleType("antenv.axon_hooks")
        _hooks._hook = None
        _hooks.get_axon_ntff_profile_hook = lambda: _hooks._hook

        def _set_hook(h):
            _hooks._hook = h

        _hooks.set_axon_ntff_profile_hook = _set_hook
        sys.modules["antenv.axon_hooks"] = _hooks
        antenv.axon_hooks = _hooks
    except Exception:
        pass

B, N, D = 4, 2048, 1024
EH = 512          # e-half width
P = 128
DT = D // P       # 8 d-tiles
ET = EH // P      # 4 e-tiles of the half
NCK = 512         # n-chunk
CHUNKS = N // NCK # 4
EPS = 1e-5

_CACHE = {}
LAST_EXEC_NS = None


def _build_bass():
    import concourse.bacc as bacc
    import concourse.tile as tile
    import concourse.mybir as mybir

    f32 = mybir.dt.float32
    f32r = mybir.dt.float32r
    bf16 = mybir.dt.bfloat16
    Act = mybir.ActivationFunctionType
    Alu = mybir.AluOpType

    nc = bacc.Bacc()

    xrt_d = nc.dram_tensor("xrt", [D, N], f32, kind="ExternalInput")
    xit_d = nc.dram_tensor("xit", [D, N], f32, kind="ExternalInput")
    wqr_d = nc.dram_tensor("wqr", [D, D], f32, kind="ExternalInput")
    wqi_d = nc.dram_tensor("wqi", [D, D], f32, kind="ExternalInput")
    wk_d = nc.dram_tensor("wk", [D, D], f32, kind="ExternalInput")
    wv_d = nc.dram_tensor("wv", [D, EH], f32, kind="ExternalInput")
    bias_d = nc.dram_tensor("bias", [EH], f32, kind="ExternalInput")
    out_d = nc.dram_tensor("out_t", [EH, N], f32, kind="ExternalOutput")

    xrt_r = xrt_d.rearrange("(t p) n -> p t n", p=P)
    xit_r = xit_d.rearrange("(t p) n -> p t n", p=P)
    wqr_r = wqr_d.rearrange("(t p) e -> p t e", p=P)
    wqi_r = wqi_d.rearrange("(t p) e -> p t e", p=P)
    wk_r = wk_d.rearrange("(t p) e -> p t e", p=P)
    wv_r = wv_d.rearrange("(t p) e -> p t e", p=P)
    bias_r = bias_d.rearrange("(t p) -> p t", p=P)
    out_r = out_d.rearrange("(t p) n -> p t n", p=P)

    with tile.TileContext(nc) as tc:
        with tc.tile_pool(name="outer", bufs=1) as outer:
            a_sb = outer.tile([P, DT, EH], f32, tag="a_sb")
            skinv = outer.tile([P, DT], f32, tag="skinv")
            svinv = outer.tile([P, ET], f32, tag="svinv")
            bias_pp = outer.tile([P, ET], f32, tag="bias_pp")
            zero32 = outer.tile([P, 1], f32, tag="zero32")
            ones_bf = outer.tile([P, 1], bf16, tag="ones_bf")
            nc.vector.memset(zero32[:], 0.0)
            nc.vector.memset(ones_bf[:], 1.0)
            nc.sync.dma_start(out=bias_pp[:], in_=bias_r)

            sk2_sb = outer.tile([P, DT], f32, tag="sk2_sb")
            sv2_sb = outer.tile([P, ET], f32, tag="sv2_sb")
            nc.vector.memset(sk2_sb[:], 0.0)
            nc.vector.memset(sv2_sb[:], 0.0)

            # ---------------- Phase A: K, V, A = K^T V, norms ----------------
            NCKA = 256
            CHA = N // NCKA
            NTA = NCKA // P
            ctx_wq = tc.tile_pool(name="wq", bufs=1)
            wq = ctx_wq.__enter__()
            wqr_sb = wq.tile([P, DT, D], f32r, tag="wqr")
            wqi_sb = wq.tile([P, DT, D], f32r, tag="wqi")
            with (
                tc.tile_pool(name="wkv", bufs=1) as wkv,
                tc.tile_pool(name="xin", bufs=2) as xin,
                tc.tile_pool(name="kv", bufs=2) as kvp,
                tc.tile_pool(name="sqp", bufs=4) as sqp,
                tc.tile_pool(name="ps_kv", bufs=3, space="PSUM") as ps_kv,
                tc.tile_pool(name="ps_a", bufs=2, space="PSUM") as ps_a,
                tc.tile_pool(name="nrm_ps", bufs=3, space="PSUM") as nrm_ps,
            ):
                wk_sb = wkv.tile([P, DT, D], f32r, tag="wk")
                wv_sb = wkv.tile([P, DT, EH], f32r, tag="wv")

                for c4 in range(CHA):
                    ns = c4 * NCKA
                    xrt_c = xin.tile([P, DT, NCKA], f32r, tag="xrt_c")
                    xit_c = xin.tile([P, DT, NCKA], f32r, tag="xit_c")
                    if c4 == 0:
                        # startup: split the K-path inputs (xrt + wk) across
                        # both DMA queues so the first chains unblock fastest
                        nc.sync.dma_start(out=xrt_c[:],
                                          in_=xrt_r[:, :, ns:ns + NCKA].bitcast(f32r))
                        for t in range(DT):
                            eng = nc.sync if t % 2 else nc.gpsimd
                            eng.dma_start(out=wk_sb[:, t, :],
                                          in_=wk_r[:, t, :].bitcast(f32r))
                        nc.gpsimd.dma_start(out=xit_c[:],
                                            in_=xit_r[:, :, ns:ns + NCKA].bitcast(f32r))
                        for t in range(DT):
                            eng = nc.gpsimd if t % 2 else nc.sync
                            eng.dma_start(out=wv_sb[:, t, :],
                                          in_=wv_r[:, t, :].bitcast(f32r))
                    else:
                        nc.sync.dma_start(out=xrt_c[:], in_=xrt_r[:, :, ns:ns + NCKA].bitcast(f32r))
                        nc.gpsimd.dma_start(out=xit_c[:], in_=xit_r[:, :, ns:ns + NCKA].bitcast(f32r))
                    if c4 == 3:
                        # prefetch phase-C weights in the DMA slack mid-phase
                        for t in range(DT):
                            nc.sync.dma_start(out=wqr_sb[:, t, :],
                                              in_=wqr_r[:, t, :].bitcast(f32r))
                            nc.gpsimd.dma_start(out=wqi_sb[:, t, :],
                                                in_=wqi_r[:, t, :].bitcast(f32r))

                    k_c = kvp.tile([P, NTA, D], f32r, tag="k_c")
                    v_c = kvp.tile([P, NTA, EH], f32r, tag="v_c")

                    first = c4 == 0
                    for nt in range(NTA):
                        nsl = slice(nt * P, (nt + 1) * P)
                        sqs = []
                        for do2 in range(2):
                            kps = ps_kv.tile([P, 512], f32, tag="kvps")
                            for t in range(DT):
                                nc.tensor.matmul(
                                    kps[:], xrt_c[:, t, nsl],
                                    wk_sb[:, t, do2 * 512:(do2 + 1) * 512],
                                    start=(t == 0), stop=(t == DT - 1))
                            nc.vector.tensor_copy(
                                out=k_c[:, nt, do2 * 512:(do2 + 1) * 512], in_=kps[:])
                            sq = sqp.tile([P, 512], bf16, tag="sq")
                            nc.scalar.activation(out=sq[:], in_=kps[:], func=Act.Square,
                                                 bias=zero32[:], scale=1.0)
                            sqs.append(sq)
                        vps = ps_kv.tile([P, 512], f32, tag="kvps")
                        for t in range(DT):
                            nc.tensor.matmul(vps[:], xit_c[:, t, nsl], wv_sb[:, t, :],
                                             start=(t == 0), stop=(t == DT - 1))
                        nc.vector.tensor_copy(out=v_c[:, nt, :], in_=vps[:])
                        sqv = sqp.tile([P, 512], bf16, tag="sq")
                        nc.scalar.activation(out=sqv[:], in_=vps[:], func=Act.Square,
                                             bias=zero32[:], scale=1.0)
                        # batched norm matmuls: one pipeline break per n-tile
                        for do2 in range(2):
                            nps = nrm_ps.tile([P, 4], f32, tag="nps")
                            for j in range(4):
                                nc.tensor.matmul(
                                    nps[:, j:j + 1], sqs[do2][:, j * P:(j + 1) * P],
                                    ones_bf[:], start=True, stop=True)
                            nc.vector.tensor_add(
                                out=sk2_sb[:, do2 * 4:(do2 + 1) * 4],
                                in0=sk2_sb[:, do2 * 4:(do2 + 1) * 4], in1=nps[:])
                        npsv = nrm_ps.tile([P, 4], f32, tag="nps")
                        for j in range(4):
                            nc.tensor.matmul(
                                npsv[:, j:j + 1], sqv[:, j * P:(j + 1) * P],
                                ones_bf[:], start=True, stop=True)
                        nc.vector.tensor_add(out=sv2_sb[:], in0=sv2_sb[:], in1=npsv[:])

                    # A += K_c^T @ V_c  (contract the chunk's rows)
                    for dt in range(DT):
                        aps = ps_a.tile([P, EH], f32, tag="aps")
                        for nt in range(NTA):
                            nc.tensor.matmul(aps[:], k_c[:, nt, dt * P:(dt + 1) * P],
                                             v_c[:, nt, :], start=(nt == 0),
                                             stop=(nt == NTA - 1))
                        if first:
                            nc.vector.tensor_copy(out=a_sb[:, dt, :].bitcast(f32r),
                                                  in_=aps[:])
                        else:
                            nc.vector.tensor_add(out=a_sb[:, dt, :].bitcast(f32r),
                                                 in0=a_sb[:, dt, :], in1=aps[:])

            # ---------------- Phase B: finalize norms, scale A ----------------
            nc.scalar.activation(out=skinv[:], in_=sk2_sb[:], func=Act.Sqrt,
                                 bias=zero32[:], scale=1.0)
            nc.vector.tensor_scalar_max(skinv[:], skinv[:], EPS)
            nc.vector.reciprocal(skinv[:], skinv[:])
            nc.scalar.activation(out=svinv[:], in_=sv2_sb[:], func=Act.Sqrt,
                                 bias=zero32[:], scale=1.0)
            nc.vector.tensor_scalar_max(svinv[:], svinv[:], EPS)
            nc.vector.reciprocal(svinv[:], svinv[:])
            for dt in range(DT):
                nc.vector.tensor_scalar_mul(
                    out=a_sb[:, dt, :].bitcast(f32r), in0=a_sb[:, dt, :],
                    scalar1=skinv[:, dt:dt + 1])

            # ---------------- Phase C: Q^T and out^T = (A')^T Q^T ------------
            with (
                tc.tile_pool(name="xin2", bufs=2) as xin2,
                tc.tile_pool(name="qrp", bufs=3) as qrp,
                tc.tile_pool(name="qtp", bufs=2) as qtp,
                tc.tile_pool(name="outp", bufs=3) as outp,
                tc.tile_pool(name="ps_q", bufs=4, space="PSUM") as ps_q,
                tc.tile_pool(name="ps_o", bufs=2, space="PSUM") as ps_o,
            ):
                for c4 in range(CHUNKS):
                    ns = c4 * NCK
                    xrt_c = xin2.tile([P, DT, NCK], f32r, tag="xrt_c2")
                    xit_c = xin2.tile([P, DT, NCK], f32r, tag="xit_c2")
                    nc.sync.dma_start(out=xrt_c[:], in_=xrt_r[:, :, ns:ns + NCK].bitcast(f32r))
                    nc.gpsimd.dma_start(out=xit_c[:], in_=xit_r[:, :, ns:ns + NCK].bitcast(f32r))

                    qt_c = qtp.tile([P, DT, NCK], f32r, tag="qt_c")
                    for dqt in range(DT):
                        qsl = slice(dqt * P, (dqt + 1) * P)
                        qrps = ps_q.tile([P, NCK], f32, tag="qps")
                        for t in range(DT):
                            nc.tensor.matmul(qrps[:], wqr_sb[:, t, qsl], xrt_c[:, t, :],
                                             start=(t == 0), stop=(t == DT - 1))
                        qr_sb = qrp.tile([P, NCK], f32, tag="qr_sb")
                        nc.scalar.activation(out=qr_sb[:], in_=qrps[:], func=Act.Copy,
                                             bias=0.0, scale=1.0)
                        qips = ps_q.tile([P, NCK], f32, tag="qps")
                        for t in range(DT):
                            nc.tensor.matmul(qips[:], wqi_sb[:, t, qsl], xit_c[:, t, :],
                                             start=(t == 0), stop=(t == DT - 1))
                        nc.vector.tensor_mul(out=qt_c[:, dqt, :], in0=qips[:], in1=qr_sb[:])

                    for et in range(ET):
                        esl = slice(et * P, (et + 1) * P)
                        ops_t = ps_o.tile([P, NCK], f32, tag="ops")
                        for dt in range(DT):
                            nc.tensor.matmul(ops_t[:], a_sb[:, dt, esl].bitcast(f32r),
                                             qt_c[:, dt, :],
                                             start=(dt == 0), stop=(dt == DT - 1))
                        out_sb = outp.tile([P, NCK], f32, tag="out_sb")
                        nc.vector.tensor_scalar(
                            out=out_sb[:], in0=ops_t[:],
                            scalar1=svinv[:, et:et + 1], scalar2=bias_pp[:, et:et + 1],
                            op0=Alu.mult, op1=Alu.add)
                        nc.sync.dma_start(out=out_r[:, et, ns:ns + NCK], in_=out_sb[:])

            ctx_wq.__exit__(None, None, None)

    nc.finalize()
    return nc


def kernel(x_real, x_imag, w_query_real, w_query_imag, w_key, w_value, bias):
    global LAST_EXEC_NS
    from concourse.bass_utils import run_bass_kernel_spmd

    x_real = np.ascontiguousarray(np.asarray(x_real, dtype=np.float32))
    x_imag = np.ascontiguousarray(np.asarray(x_imag, dtype=np.float32))
    wqr = np.ascontiguousarray(np.asarray(w_query_real, dtype=np.float32))
    wqi = np.ascontiguousarray(np.asarray(w_query_imag, dtype=np.float32))
    wk = np.ascontiguousarray(np.asarray(w_key, dtype=np.float32))
    wv = np.ascontiguousarray(np.asarray(w_value, dtype=np.float32))
    bias = np.ascontiguousarray(np.asarray(bias, dtype=np.float32))

    nc = _CACHE.get("nc")
    if nc is None:
        nc = _build_bass()
        _CACHE["nc"] = nc

    xrt = [np.ascontiguousarray(x_real[b].T) for b in range(B)]
    xit = [np.ascontiguousarray(x_imag[b].T) for b in range(B)]
    wv_h = [np.ascontiguousarray(wv[:, eh * EH:(eh + 1) * EH]) for eh in range(2)]
    bias_h = [np.ascontiguousarray(bias[eh * EH:(eh + 1) * EH]) for eh in range(2)]

    in_maps = []
    for c in range(8):
        b, eh = c // 2, c % 2
        in_maps.append({
            "xrt": xrt[b], "xit": xit[b],
            "wqr": wqr, "wqi": wqi, "wk": wk,
            "wv": wv_h[eh], "bias": bias_h[eh],
        })

    res = run_bass_kernel_spmd(nc, in_maps, list(range(8)))
    LAST_EXEC_NS = res.exec_time_ns

    out = np.empty((B, N, D), dtype=np.float32)
    for c in range(8):
        b, eh = c // 2, c % 2
        out[b, :, eh * EH:(eh + 1) * EH] = np.asarray(res.results[c]["out_t"]).T
    return out
